# revision 11
# baseline (speedup 1.0000x reference)
"""Chamfer loss kernel for Trainium2, 8 NeuronCores (SPMD data-parallel).

Strategy (data-parallel over selected pairs, per the sharding hint):
  - Host: dedupe the (batch, seed) pairs in idx (weights = multiplicities).
    Each pair contributes two direction-units (x->nearest-y and
    y->nearest-x); the 2U units are distributed round-robin over 8 cores.
  - Per unit the 2048 query points are sorted into 16 spatial blocks of
    128 (median-cut k-d splits on the widest axis). For each block the
    host selects the C database points nearest to the block's bounding
    box (rank by squared clamp-distance). The device computes the dense
    [128 queries x C candidates] squared-distance tile per block with a
    K=16 matmul trick (bf16 hi/lo split products, fp32-quality):
      k 0..2 : xh_d * ah_d   (a = -2y)     k 9..11: xl_d * al_d
      k 3..5 : xh_d * al_d                 k 12/13: rxh/rxl * 1
      k 6..8 : xl_d * ah_d                 k 14/15: 1 * ryh/ryl
    Eight blocks are packed into one K=128 matmul group (block-diagonal
    rhs: block s occupies K-rows 16s..16s+16 and its own column range;
    off-diagonal zeros are memset once per SBUF buffer and only the
    diagonal slabs are DMA'd) - K=128 streams PE columns ~2x faster
    than K=16 and amortizes weight loads.
  - Min-reduction per group: ACT (+DVE for the tail blocks) evacuates
    PSUM fp32 -> SBUF f16, DVE tensor_tensor folds halve the candidate
    axis 3x, one DVE tensor_reduce yields [128, 16] per-query mins.
  - Host exactness certificate per query: every non-candidate point d
    satisfies dist(q, d) >= depth(q) + rho, where rho is the smallest
    excluded clamp-distance and depth is q's distance to its block's
    bbox boundary (valid when all bbox-interior points are candidates,
    i.e. rho > 0). Queries whose device min exceeds the certificate are
    recomputed exactly on host (cKDTree when available). Weighted
    sum / num.
"""

import numpy as np
import ml_dtypes
from contextlib import ExitStack

import concourse.bacc as bacc
import concourse.tile as tile
from concourse import mybir
from concourse.bass_utils import run_bass_kernel_spmd

N_CORES = 8
NPTS = 2048
NBLK = 16          # query blocks per unit (128 queries each)
CAND = 192         # candidates per block
NACT = 7           # blocks per group evacuated by ACT (rest: DVE copy)
GCOL = 8 * CAND    # columns per K=128 matmul group
BF16 = ml_dtypes.bfloat16
F16 = mybir.dt.float16
F32 = mybir.dt.float32
MIN = mybir.AluOpType.min

_BUILD_CACHE = {}


def build_program(n_slots: int, repeats: int = 1, stages: str = "full"):
    """Build + compile the per-core bass program for n_slots units."""
    key = (n_slots, repeats, stages)
    if key in _BUILD_CACHE:
        return _BUILD_CACHE[key]

    c = CAND
    nmm = (GCOL + 511) // 512  # FD<=512 matmul chunks per group
    nc = bacc.Bacc(
        "TRN2", target_bir_lowering=False, debug=False, num_devices=N_CORES
    )
    w_ap = nc.dram_tensor(
        "w", [n_slots, 128, 256], mybir.dt.bfloat16, kind="ExternalInput"
    ).ap()
    r_ap = nc.dram_tensor(
        "r", [n_slots, NBLK, 16, c], mybir.dt.bfloat16, kind="ExternalInput"
    ).ap()
    o_ap = nc.dram_tensor(
        "o", [n_slots, 128, NBLK], F32, kind="ExternalOutput"
    ).ap()

    with tile.TileContext(nc) as tc:
        with ExitStack() as ctx:
            in_pool = ctx.enter_context(tc.tile_pool(name="inp", bufs=3))
            conv_pool = ctx.enter_context(tc.tile_pool(name="conv", bufs=3))
            fold_pool = ctx.enter_context(tc.tile_pool(name="fold", bufs=2))
            f3_pool = ctx.enter_context(tc.tile_pool(name="f3", bufs=2))
            out_pool = ctx.enter_context(tc.tile_pool(name="outp", bufs=3))
            mm_psum = ctx.enter_context(
                tc.tile_pool(name="mmps", bufs=2, space="PSUM")
            )

            # R tiles carry the block-diagonal zeros persistently: zero
            # all in_pool 'rt' rotations once; unit DMAs rewrite only the
            # diagonal slabs.
            for _ in range(3):
                rt0 = in_pool.tile([128, 2, GCOL], mybir.dt.bfloat16, tag="rt")
                nc.vector.memset(rt0[:], 0.0)

            def body():
                for s in range(n_slots):
                    wt = in_pool.tile([128, 256], mybir.dt.bfloat16, tag="wt")
                    rt = in_pool.tile([128, 2, GCOL], mybir.dt.bfloat16, tag="rt")
                    nc.gpsimd.dma_start(wt[:], w_ap[s])
                    for blk in range(NBLK):
                        g, sb = blk // 8, blk % 8
                        nc.gpsimd.dma_start(
                            rt[16 * sb : 16 * sb + 16, g, c * sb : c * (sb + 1)],
                            r_ap[s, blk],
                        )

                    outt = out_pool.tile([128, NBLK], F32)
                    f3u = f3_pool.tile([128, NBLK, c // 8], F16, tag="f3")

                    for g in range(2):
                        ps = mm_psum.tile([128, nmm, 512], F32, tag="ps")
                        psf = ps[:].rearrange("p a b -> p (a b)")
                        for j in range(nmm):
                            lo = 512 * j
                            hi = min(GCOL, lo + 512)
                            nc.tensor.matmul(
                                psf[:, lo:hi],
                                lhsT=wt[:, 128 * g : 128 * (g + 1)],
                                rhs=rt[:, g, lo:hi],
                                start=True,
                                stop=True,
                            )
                        if stages == "mm":
                            nc.scalar.activation(
                                out=outt[:, 8 * g : 8 * g + 8],
                                in_=psf[:, 0:8],
                                func=mybir.ActivationFunctionType.Copy,
                            )
                            continue
                        cp = conv_pool.tile([128, 8, c], F16, tag="cp")
                        cpf = cp[:].rearrange("p a b -> p (a b)")
                        nc.scalar.activation(
                            out=cpf[:, 0 : NACT * c],
                            in_=psf[:, 0 : NACT * c],
                            func=mybir.ActivationFunctionType.Copy,
                        )
                        if NACT < 8:
                            nc.vector.tensor_copy(
                                cpf[:, NACT * c : 8 * c], psf[:, NACT * c : 8 * c]
                            )
                        f1 = fold_pool.tile([128, 8, c // 2], F16, tag="f1")
                        nc.vector.tensor_tensor(
                            f1[:], cp[:, :, 0 : c // 2], cp[:, :, c // 2 : c], MIN
                        )
                        f2 = fold_pool.tile([128, 8, c // 4], F16, tag="f2")
                        nc.vector.tensor_tensor(
                            f2[:], f1[:, :, 0 : c // 4], f1[:, :, c // 4 : c // 2], MIN
                        )
                        nc.vector.tensor_tensor(
                            f3u[:, 8 * g : 8 * g + 8, :],
                            f2[:, :, 0 : c // 8],
                            f2[:, :, c // 8 : c // 4],
                            MIN,
                        )

                    if stages == "full":
                        nc.vector.tensor_reduce(
                            out=outt[:],
                            in_=f3u[:],
                            axis=mybir.AxisListType.X,
                            op=MIN,
                        )
                    nc.gpsimd.dma_start(o_ap[s], outt[:])

            if repeats == 1:
                body()
            else:
                with tc.For_i(0, repeats, 1):
                    body()

    nc.compile()
    _BUILD_CACHE[key] = nc
    return nc


def _split_bf16(x: np.ndarray):
    hi = x.astype(BF16)
    lo = (x - hi.astype(np.float32)).astype(BF16)
    return hi, lo


def _make_w(qs: np.ndarray) -> np.ndarray:
    """qs: [3, 2048] fp32 sorted queries -> W [16, 2048] bf16."""
    n = qs.shape[1]
    rx = (qs * qs).sum(axis=0)
    xh, xl = _split_bf16(qs)
    rxh, rxl = _split_bf16(rx)
    W = np.empty((16, n), dtype=BF16)
    W[0:3] = xh
    W[3:6] = xh
    W[6:9] = xl
    W[9:12] = xl
    W[12] = rxh
    W[13] = rxl
    W[14:16] = np.ones((2, n), dtype=BF16)
    return W


def _make_r(dc: np.ndarray) -> np.ndarray:
    """dc: [NBLK, C, 3] fp32 candidate coords -> R [NBLK, 16, C] bf16."""
    nb, cc, _ = dc.shape
    y = dc.reshape(-1, 3).T  # [3, NBLK*C]
    a = -2.0 * y
    ry = (y * y).sum(axis=0)
    ah, al = _split_bf16(a)
    ryh, ryl = _split_bf16(ry)
    n = y.shape[1]
    R = np.empty((16, n), dtype=BF16)
    R[0:3] = ah
    R[3:6] = al
    R[6:9] = ah
    R[9:12] = al
    R[12:14] = np.ones((2, n), dtype=BF16)
    R[14] = ryh
    R[15] = ryl
    return R.reshape(16, nb, cc).transpose(1, 0, 2)


def _kd_order(Q: np.ndarray) -> np.ndarray:
    """Median-cut widest-axis splits of Q [N,3] into NBLK groups of equal
    size; returns the concatenated index order (block-major)."""
    groups = [np.arange(Q.shape[0])]
    while len(groups) < NBLK:
        new = []
        for g in groups:
            pts = Q[g]
            ax = int(np.argmax(pts.max(0) - pts.min(0)))
            o = g[np.argsort(Q[g, ax], kind="stable")]
            h = len(o) // 2
            new.append(o[:h])
            new.append(o[h:])
        groups = new
    return np.concatenate(groups)


def prepare_inputs(preds: np.ndarray, gts: np.ndarray, idx: np.ndarray):
    """Dedupe pairs, build per-core input maps + certificate metadata.

    Returns (in_maps, plan, S, num). plan entries:
      (cnt, core, slot, Qs [2048,3] f32, D [2048,3] f32,
       rho2 [NBLK] f64, depth [NBLK,128] f64)
    """
    preds = np.asarray(preds, dtype=np.float32)
    gts = np.asarray(gts, dtype=np.float32)
    idx = np.asarray(idx)
    num = idx.shape[0]

    uniq = {}
    for row in idx:
        key = (int(row[0]), int(row[1]))
        uniq[key] = uniq.get(key, 0) + 1
    pairs = list(uniq.items())
    n_units = 2 * len(pairs)
    S = (n_units + N_CORES - 1) // N_CORES

    W_all = np.zeros((N_CORES, S, 128, 256), dtype=BF16)
    R_all = np.zeros((N_CORES, S, NBLK, 16, CAND), dtype=BF16)
    plan = []
    u = 0
    for (b, sd), cnt in pairs:
        X = preds[b, :, :, sd].T  # [2048, 3]
        Y = gts[b].T              # [2048, 3]
        for Q, D in ((X, Y), (Y, X)):
            order = _kd_order(Q)
            Qs = Q[order]                          # [2048, 3] block-major
            blocks = Qs.reshape(NBLK, 128, 3)
            lo = blocks.min(axis=1)                # [NBLK, 3]
            hi = blocks.max(axis=1)
            clamped = np.clip(D[None, :, :], lo[:, None, :], hi[:, None, :])
            bbd = ((D[None, :, :] - clamped) ** 2).sum(-1)  # [NBLK, 2048]
            part = np.argpartition(bbd, CAND, axis=1)
            cand = part[:, :CAND]                  # [NBLK, CAND]
            rho2 = np.take_along_axis(bbd, part[:, CAND : CAND + 1], axis=1)[:, 0]
            depth = np.minimum(blocks - lo[:, None, :], hi[:, None, :] - blocks).min(
                axis=2
            )  # [NBLK, 128]

            core, slot = u % N_CORES, u // N_CORES
            # K=128 pack: W16 [16, 2048] -> [krow, g, s, m] -> group layout
            # [row=16*s+krow, col=128*g+m]
            W16 = _make_w(Qs.T).reshape(16, 2, 8, 128)
            W_all[core, slot] = (
                W16.transpose(2, 0, 1, 3)          # [s, krow, g, m]
                .reshape(128, 2, 128)
                .reshape(128, 256)
            )
            R_all[core, slot] = _make_r(
                np.take_along_axis(D[None, :, :], cand[:, :, None], axis=1)
            )
            plan.append(
                (cnt, core, slot, Qs, D, rho2.astype(np.float64),
                 depth.astype(np.float64))
            )
            u += 1

    in_maps = [{"w": W_all[c], "r": R_all[c]} for c in range(N_CORES)]
    return in_maps, plan, S, num


def _exact_min_sq(queries: np.ndarray, D: np.ndarray) -> np.ndarray:
    """Exact squared nn distance of each query against D (host fixup)."""
    try:
        from scipy.spatial import cKDTree
    except Exception:
        out = np.empty(queries.shape[0])
        for i in range(0, queries.shape[0], 512):
            q = queries[i : i + 512]
            d2 = ((q[:, None, :] - D[None, :, :]) ** 2).sum(-1)
            out[i : i + 512] = d2.min(axis=1)
        return out
    tree = cKDTree(D)
    dd, _ = tree.query(queries)
    return dd ** 2


def finish(results, plan, num):
    total = 0.0
    for cnt, core, slot, Qs, D, rho2, depth in plan:
        o = results[core]["o"][slot]          # [128, NBLK] f32
        m = o.T.astype(np.float64)            # [NBLK, 128] block-major mins
        cert = (depth + np.sqrt(np.maximum(rho2, 0.0))[:, None]) ** 2
        suspect = (m >= cert * 0.999) | (rho2 <= 0.0)[:, None]
        if suspect.any():
            qs = Qs.reshape(NBLK, 128, 3)[suspect]
            m[suspect] = _exact_min_sq(qs.astype(np.float64), D.astype(np.float64))
        total += cnt * m.sum()
    return np.float32(total / num)


def kernel(preds, gts, idx):
    in_maps, plan, S, num = prepare_inputs(preds, gts, idx)
    nc = build_program(S)
    res = run_bass_kernel_spmd(nc, in_maps, list(range(N_CORES)))
    return finish(res.results, plan, num)


# revision 15
# speedup vs baseline: 2.5493x; 2.5493x over previous
"""Chamfer loss kernel for Trainium2, 8 NeuronCores (SPMD data-parallel).

Strategy (data-parallel over selected pairs, per the sharding hint):
  - Host: dedupe the (batch, seed) pairs in idx (weights = multiplicities).
    Each pair contributes two direction-units (x->nearest-y and
    y->nearest-x); the 2U units are distributed round-robin over 8 cores.
  - Per unit the 2048 query points are sorted into 16 spatial blocks of
    128 (median-cut k-d splits on the widest axis). For each block the
    host selects the C database points nearest to the block's bounding
    box (rank by squared clamp-distance). The device computes the dense
    [128 queries x C candidates] squared-distance tile per block with a
    K=16 matmul trick (bf16 hi/lo split products, fp32-quality):
      k 0..2 : xh_d * ah_d   (a = -2y)     k 9..11: xl_d * al_d
      k 3..5 : xh_d * al_d                 k 12/13: rxh/rxl * 1
      k 6..8 : xl_d * ah_d                 k 14/15: 1 * ryh/ryl
    Eight blocks are packed into one K=128 matmul group (block-diagonal
    rhs: block s occupies K-rows 16s..16s+16 and its own column range;
    off-diagonal zeros are memset once per SBUF buffer and only the
    diagonal slabs are DMA'd) - K=128 streams PE columns ~2x faster
    than K=16 and amortizes weight loads.
  - Min-reduction per group: ACT (+DVE for the tail blocks) evacuates
    PSUM fp32 -> SBUF f16, DVE tensor_tensor folds halve the candidate
    axis 3x, one DVE tensor_reduce yields [128, 16] per-query mins.
  - Host exactness certificate per query: every non-candidate point d
    satisfies dist(q, d) >= depth(q) + rho, where rho is the smallest
    excluded clamp-distance and depth is q's distance to its block's
    bbox boundary (valid when all bbox-interior points are candidates,
    i.e. rho > 0). Queries whose device min exceeds the certificate are
    recomputed exactly on host (cKDTree when available). Weighted
    sum / num.
"""

import numpy as np
import ml_dtypes
from contextlib import ExitStack

import concourse.bacc as bacc
import concourse.tile as tile
from concourse import mybir
from concourse.bass_utils import run_bass_kernel_spmd

N_CORES = 8
NPTS = 2048
NBLK = 16          # query blocks per unit (128 queries each)
CAND = 192         # candidates per block
NACT = 7           # blocks per group evacuated by ACT (rest: DVE copy)
GCOL = 8 * CAND    # columns per K=128 matmul group
BF16 = ml_dtypes.bfloat16
F16 = mybir.dt.float16
F32 = mybir.dt.float32
MIN = mybir.AluOpType.min

_BUILD_CACHE = {}


def build_program(n_slots: int, repeats: int = 1, stages: str = "full"):
    """Build + compile the per-core bass program for n_slots units."""
    key = (n_slots, repeats, stages)
    if key in _BUILD_CACHE:
        return _BUILD_CACHE[key]

    c = CAND
    nmm = (GCOL + 511) // 512  # FD<=512 matmul chunks per group
    nc = bacc.Bacc(
        "TRN2", target_bir_lowering=False, debug=False, num_devices=N_CORES
    )
    w_ap = nc.dram_tensor(
        "w", [n_slots, 128, 256], mybir.dt.bfloat16, kind="ExternalInput"
    ).ap()
    r_ap = nc.dram_tensor(
        "r", [n_slots, 128, 2, GCOL], mybir.dt.bfloat16, kind="ExternalInput"
    ).ap()
    o_ap = nc.dram_tensor(
        "o", [n_slots, 128, NBLK], F32, kind="ExternalOutput"
    ).ap()

    with tile.TileContext(nc) as tc:
        with ExitStack() as ctx:
            in_pool = ctx.enter_context(tc.tile_pool(name="inp", bufs=3))
            conv_pool = ctx.enter_context(tc.tile_pool(name="conv", bufs=3))
            fold_pool = ctx.enter_context(tc.tile_pool(name="fold", bufs=2))
            f3_pool = ctx.enter_context(tc.tile_pool(name="f3", bufs=2))
            out_pool = ctx.enter_context(tc.tile_pool(name="outp", bufs=3))
            mm_psum = ctx.enter_context(
                tc.tile_pool(name="mmps", bufs=2, space="PSUM")
            )

            def body():
                for s in range(n_slots):
                    wt = in_pool.tile([128, 256], mybir.dt.bfloat16, tag="wt")
                    rt = in_pool.tile([128, 2, GCOL], mybir.dt.bfloat16, tag="rt")
                    nc.gpsimd.dma_start(wt[:], w_ap[s])
                    nc.sync.dma_start(rt[:], r_ap[s])

                    outt = out_pool.tile([128, NBLK], F32)
                    f3u = f3_pool.tile([128, NBLK, c // 8], F16, tag="f3")

                    for g in range(2):
                        ps = mm_psum.tile([128, nmm, 512], F32, tag="ps")
                        psf = ps[:].rearrange("p a b -> p (a b)")
                        for j in range(nmm):
                            lo = 512 * j
                            hi = min(GCOL, lo + 512)
                            nc.tensor.matmul(
                                psf[:, lo:hi],
                                lhsT=wt[:, 128 * g : 128 * (g + 1)],
                                rhs=rt[:, g, lo:hi],
                                start=True,
                                stop=True,
                            )
                        if stages == "mm":
                            nc.scalar.activation(
                                out=outt[:, 8 * g : 8 * g + 8],
                                in_=psf[:, 0:8],
                                func=mybir.ActivationFunctionType.Copy,
                            )
                            continue
                        cp = conv_pool.tile([128, 8, c], F16, tag="cp")
                        cpf = cp[:].rearrange("p a b -> p (a b)")
                        nc.scalar.activation(
                            out=cpf[:, 0 : NACT * c],
                            in_=psf[:, 0 : NACT * c],
                            func=mybir.ActivationFunctionType.Copy,
                        )
                        if NACT < 8:
                            nc.vector.tensor_copy(
                                cpf[:, NACT * c : 8 * c], psf[:, NACT * c : 8 * c]
                            )
                        f1 = fold_pool.tile([128, 8, c // 2], F16, tag="f1")
                        nc.vector.tensor_tensor(
                            f1[:], cp[:, :, 0 : c // 2], cp[:, :, c // 2 : c], MIN
                        )
                        f2 = fold_pool.tile([128, 8, c // 4], F16, tag="f2")
                        nc.vector.tensor_tensor(
                            f2[:], f1[:, :, 0 : c // 4], f1[:, :, c // 4 : c // 2], MIN
                        )
                        nc.vector.tensor_tensor(
                            f3u[:, 8 * g : 8 * g + 8, :],
                            f2[:, :, 0 : c // 8],
                            f2[:, :, c // 8 : c // 4],
                            MIN,
                        )

                    if stages == "full":
                        nc.vector.tensor_reduce(
                            out=outt[:],
                            in_=f3u[:],
                            axis=mybir.AxisListType.X,
                            op=MIN,
                        )
                    nc.gpsimd.dma_start(o_ap[s], outt[:])

            if repeats == 1:
                body()
            else:
                with tc.For_i(0, repeats, 1):
                    body()

    nc.compile()
    _BUILD_CACHE[key] = nc
    return nc


def _split_bf16(x: np.ndarray):
    hi = x.astype(BF16)
    lo = (x - hi.astype(np.float32)).astype(BF16)
    return hi, lo


def _make_w(qs: np.ndarray) -> np.ndarray:
    """qs: [3, 2048] fp32 sorted queries -> W [16, 2048] bf16."""
    n = qs.shape[1]
    rx = (qs * qs).sum(axis=0)
    xh, xl = _split_bf16(qs)
    rxh, rxl = _split_bf16(rx)
    W = np.empty((16, n), dtype=BF16)
    W[0:3] = xh
    W[3:6] = xh
    W[6:9] = xl
    W[9:12] = xl
    W[12] = rxh
    W[13] = rxl
    W[14:16] = np.ones((2, n), dtype=BF16)
    return W


def _make_r(dc: np.ndarray) -> np.ndarray:
    """dc: [NBLK, C, 3] fp32 candidate coords -> R [NBLK, 16, C] bf16."""
    nb, cc, _ = dc.shape
    y = dc.reshape(-1, 3).T  # [3, NBLK*C]
    a = -2.0 * y
    ry = (y * y).sum(axis=0)
    ah, al = _split_bf16(a)
    ryh, ryl = _split_bf16(ry)
    n = y.shape[1]
    R = np.empty((16, n), dtype=BF16)
    R[0:3] = ah
    R[3:6] = al
    R[6:9] = ah
    R[9:12] = al
    R[12:14] = np.ones((2, n), dtype=BF16)
    R[14] = ryh
    R[15] = ryl
    return R.reshape(16, nb, cc).transpose(1, 0, 2)


def _kd_order(Q: np.ndarray) -> np.ndarray:
    """Median-cut widest-axis splits of Q [N,3] into NBLK groups of equal
    size; returns the concatenated index order (block-major)."""
    groups = [np.arange(Q.shape[0])]
    while len(groups) < NBLK:
        new = []
        for g in groups:
            pts = Q[g]
            ax = int(np.argmax(pts.max(0) - pts.min(0)))
            o = g[np.argsort(Q[g, ax], kind="stable")]
            h = len(o) // 2
            new.append(o[:h])
            new.append(o[h:])
        groups = new
    return np.concatenate(groups)


def prepare_inputs(preds: np.ndarray, gts: np.ndarray, idx: np.ndarray):
    """Dedupe pairs, build per-core input maps + certificate metadata.

    Returns (in_maps, plan, S, num). plan entries:
      (cnt, core, slot, Qs [2048,3] f32, D [2048,3] f32,
       rho2 [NBLK] f64, depth [NBLK,128] f64)
    """
    preds = np.asarray(preds, dtype=np.float32)
    gts = np.asarray(gts, dtype=np.float32)
    idx = np.asarray(idx)
    num = idx.shape[0]

    uniq = {}
    for row in idx:
        key = (int(row[0]), int(row[1]))
        uniq[key] = uniq.get(key, 0) + 1
    pairs = list(uniq.items())
    n_units = 2 * len(pairs)
    S = (n_units + N_CORES - 1) // N_CORES

    W_all = np.zeros((N_CORES, S, 128, 256), dtype=BF16)
    R_all = np.zeros((N_CORES, S, 128, 2, GCOL), dtype=BF16)
    sidx = np.arange(8)
    plan = []
    u = 0
    for (b, sd), cnt in pairs:
        X = preds[b, :, :, sd].T  # [2048, 3]
        Y = gts[b].T              # [2048, 3]
        for Q, D in ((X, Y), (Y, X)):
            order = _kd_order(Q)
            Qs = Q[order]                          # [2048, 3] block-major
            blocks = Qs.reshape(NBLK, 128, 3)
            lo = blocks.min(axis=1)                # [NBLK, 3]
            hi = blocks.max(axis=1)
            clamped = np.clip(D[None, :, :], lo[:, None, :], hi[:, None, :])
            bbd = ((D[None, :, :] - clamped) ** 2).sum(-1)  # [NBLK, 2048]
            part = np.argpartition(bbd, CAND, axis=1)
            cand = part[:, :CAND]                  # [NBLK, CAND]
            rho2 = np.take_along_axis(bbd, part[:, CAND : CAND + 1], axis=1)[:, 0]
            depth = np.minimum(blocks - lo[:, None, :], hi[:, None, :] - blocks).min(
                axis=2
            )  # [NBLK, 128]

            core, slot = u % N_CORES, u // N_CORES
            # K=128 pack: W16 [16, 2048] -> [krow, g, s, m] -> group layout
            # [row=16*s+krow, col=128*g+m]
            W16 = _make_w(Qs.T).reshape(16, 2, 8, 128)
            W_all[core, slot] = (
                W16.transpose(2, 0, 1, 3)          # [s, krow, g, m]
                .reshape(128, 2, 128)
                .reshape(128, 256)
            )
            # R16 [NBLK, 16, C] -> block-diagonal dense [128, 2, 8C]:
            # block 8g+s occupies K-rows 16s..16s+16 of group g at columns
            # C*s..C*(s+1); elsewhere zeros.
            R16 = _make_r(
                np.take_along_axis(D[None, :, :], cand[:, :, None], axis=1)
            )
            rd = R_all[core, slot].reshape(8, 16, 2, 8, CAND)
            rd[sidx, :, 0, sidx] = R16[0:8]
            rd[sidx, :, 1, sidx] = R16[8:16]
            plan.append(
                (cnt, core, slot, Qs, D, rho2.astype(np.float64),
                 depth.astype(np.float64))
            )
            u += 1

    in_maps = [{"w": W_all[c], "r": R_all[c]} for c in range(N_CORES)]
    return in_maps, plan, S, num


def _exact_min_sq(queries: np.ndarray, D: np.ndarray) -> np.ndarray:
    """Exact squared nn distance of each query against D (host fixup)."""
    try:
        from scipy.spatial import cKDTree
    except Exception:
        out = np.empty(queries.shape[0])
        for i in range(0, queries.shape[0], 512):
            q = queries[i : i + 512]
            d2 = ((q[:, None, :] - D[None, :, :]) ** 2).sum(-1)
            out[i : i + 512] = d2.min(axis=1)
        return out
    tree = cKDTree(D)
    dd, _ = tree.query(queries)
    return dd ** 2


def finish(results, plan, num):
    total = 0.0
    for cnt, core, slot, Qs, D, rho2, depth in plan:
        o = results[core]["o"][slot]          # [128, NBLK] f32
        m = o.T.astype(np.float64)            # [NBLK, 128] block-major mins
        cert = (depth + np.sqrt(np.maximum(rho2, 0.0))[:, None]) ** 2
        suspect = (m >= cert * 0.999) | (rho2 <= 0.0)[:, None]
        if suspect.any():
            qs = Qs.reshape(NBLK, 128, 3)[suspect]
            m[suspect] = _exact_min_sq(qs.astype(np.float64), D.astype(np.float64))
        total += cnt * m.sum()
    return np.float32(total / num)


def kernel(preds, gts, idx):
    in_maps, plan, S, num = prepare_inputs(preds, gts, idx)
    nc = build_program(S)
    res = run_bass_kernel_spmd(nc, in_maps, list(range(N_CORES)))
    return finish(res.results, plan, num)
